# revision 16
# baseline (speedup 1.0000x reference)
"""Block-diagonal (per-frame) multi-head attention on 8 Trainium2 cores.

Problem: x[2,3200,512] -> QKV proj (H=8 heads, D=64) -> attention masked to
25-token frames (128 frames) -> out[2,3200,512].  N = 3200 = 128*25.

Sharding: 256 (batch, frame) groups; core c handles batch c//4, frames
(c%4)*32..+32  => 800 tokens/core, tiled as 8 x 100 tokens (4 frames).

Layout trick: host sends x pre-transposed (xT [512, 800]) so every matmul
contracts over the partition dim:
  qT/kT [feat, tok] = W.T @ xT   (lhsT = W slice, rhs = xT)
  v     [tok, feat] = xT.T @ Wv  (lhsT = xT slice, rhs = Wv)
Scores per (head, tile): S = qT_h.T @ kT_h and S^T = kT_h.T @ qT_h -- both
directly available, no transposes anywhere.  The -9e15 frame mask is rank-5
(ones + 4 frame indicators), injected by one small matmul that initializes
the PSUM accumulation group.  softmax skips max-subtraction (|scores| <~ 8).
PV uses E^T = exp(S^T) as the stationary operand with v natural as moving.
"""

import numpy as np

B, N, DIN = 2, 3200, 512
H, D = 8, 64
TL, JN = 128, 25
NCORES = 8
TOK = 800      # tokens per core
NT = 8         # token tiles per core
TT = 100       # tokens per tile (4 frames)
NEG = -9e15

# matmul dtype per stage: 'f32' | 'f32r' | 'bf16'
#   proj: QKV projection matmuls (and dtype of xT/W in SBUF+HBM)
#   qk:   dtype of qT/kT tiles (scores matmuls)
#   pv:   dtype of E^T and V tiles (PV matmul)
CONFIG = {"proj": "f32", "qk": "f32", "pv": "f32"}

_CACHE = {}
LAST_RESULT = None  # BassKernelResults of the most recent kernel() call


def _build(cfg, stage="full"):
    import concourse.bacc as bacc
    import concourse.tile as tile
    from concourse import mybir

    f32 = mybir.dt.float32
    bf16 = mybir.dt.bfloat16
    f32r = mybir.dt.float32r
    AF = mybir.ActivationFunctionType
    ALU = mybir.AluOpType
    AX = mybir.AxisListType

    def io_dt(kind):
        return bf16 if kind == "bf16" else f32

    def mm(ap, kind):
        return ap.bitcast(f32r) if kind == "f32r" else ap

    proj_dt, qk_dt, pv_dt = cfg["proj"], cfg["qk"], cfg["pv"]

    nc = bacc.Bacc("TRN2", target_bir_lowering=False, debug=False,
                   num_devices=NCORES)

    xt_d = nc.dram_tensor("xT", [DIN, TOK], io_dt(proj_dt),
                          kind="ExternalInput").ap()
    w_d = {}
    for nm in ("wq", "wk", "wv"):
        w_d[nm] = nc.dram_tensor(nm, [DIN, DIN], io_dt(proj_dt),
                                 kind="ExternalInput").ap()
    bqc_d = nc.dram_tensor("bqc", [128, 4], f32, kind="ExternalInput").ap()
    bkc_d = nc.dram_tensor("bkc", [128, 4], f32, kind="ExternalInput").ap()
    bvb_d = nc.dram_tensor("bvb", [128, DIN], f32, kind="ExternalInput").ap()
    ma_d = nc.dram_tensor("mA", [128, TT], bf16, kind="ExternalInput").ap()
    mb2_d = nc.dram_tensor("mB2", [128, 2 * TT], bf16, kind="ExternalInput").ap()
    out_d = nc.dram_tensor("out", [TOK, DIN], f32, kind="ExternalOutput").ap()

    with tile.TileContext(nc) as tc:
        with (
            tc.tile_pool(name="persist", bufs=1) as pp,
            tc.tile_pool(name="scratch", bufs=2) as sp,
        ):
            # ---- DMA in (emission order ~ priority) ----
            wq = [pp.tile([128, DIN], io_dt(proj_dt), name=f"wq{k}",
                          tag=f"wq{k}") for k in range(4)]
            xt = [pp.tile([128, TOK], io_dt(proj_dt), name=f"xt{k}",
                          tag=f"xt{k}") for k in range(4)]
            for k in range(4):
                nc.sync.dma_start(out=wq[k], in_=w_d["wq"][k * 128:(k + 1) * 128, :])
                nc.sync.dma_start(out=xt[k], in_=xt_d[k * 128:(k + 1) * 128, :])
            bqc = pp.tile([128, 4], f32, name="bqc", tag="bqc")
            bkc = pp.tile([128, 4], f32, name="bkc", tag="bkc")
            nc.sync.dma_start(out=bqc, in_=bqc_d)
            nc.sync.dma_start(out=bkc, in_=bkc_d)
            wk = [pp.tile([128, DIN], io_dt(proj_dt), name=f"wk{k}",
                          tag=f"wk{k}") for k in range(4)]
            for k in range(4):
                nc.sync.dma_start(out=wk[k], in_=w_d["wk"][k * 128:(k + 1) * 128, :])
            wv = [pp.tile([128, DIN], io_dt(proj_dt), name=f"wv{k}",
                          tag=f"wv{k}") for k in range(4)]
            for k in range(4):
                nc.sync.dma_start(out=wv[k], in_=w_d["wv"][k * 128:(k + 1) * 128, :])
            bvb = pp.tile([128, DIN], f32, name="bvb", tag="bvb")
            nc.sync.dma_start(out=bvb, in_=bvb_d)
            ma = pp.tile([128, TT], bf16, name="ma", tag="ma")
            mb2 = pp.tile([128, 2 * TT], bf16, name="mb2", tag="mb2")
            nc.sync.dma_start(out=ma, in_=ma_d)
            nc.sync.dma_start(out=mb2, in_=mb2_d)

            # ---- persistent activations ----
            qt = [pp.tile([128, TOK], io_dt(qk_dt), name=f"qt{k}",
                          tag=f"qt{k}") for k in range(4)]
            kt_ = [pp.tile([128, TOK], io_dt(qk_dt), name=f"kt{k}",
                           tag=f"kt{k}") for k in range(4)]
            vt = [pp.tile([TT, DIN], io_dt(pv_dt), name=f"vt{t}",
                          tag=f"vt{t}") for t in range(NT)]
            ot = [pp.tile([TT, DIN], f32, name=f"ot{t}", tag=f"ot{t}")
                  for t in range(NT)]

            with (
                tc.tile_pool(name="ppsum", bufs=2, space="PSUM") as pps,
                tc.tile_pool(name="vpsum", bufs=2, space="PSUM") as vps,
            ):
                # ---- q^T / k^T projections: psum[feat, tok] ----
                for (w, bc, dst) in ((wq, bqc, qt), (wk, bkc, kt_)):
                    for ft in range(4):
                        fsl = slice(ft * 128, (ft + 1) * 128)
                        for ch in range(2):
                            csl = slice(ch * 400, (ch + 1) * 400)
                            acc = pps.tile([128, 400], f32, name="pacc",
                                           tag="p", bufs=2)
                            for k in range(4):
                                nc.tensor.matmul(
                                    acc[:], mm(w[k][:, fsl], proj_dt),
                                    mm(xt[k][:, csl], proj_dt),
                                    start=(k == 0), stop=(k == 3))
                            nc.scalar.activation(dst[ft][:, csl], acc[:],
                                                 AF.Identity,
                                                 bias=bc[:, ft:ft + 1])

                # ---- v projection: psum[tok, feat]; bias+relu on DVE ----
                for t in range(NT):
                    tsl = slice(t * TT, (t + 1) * TT)
                    acc = vps.tile([TT, DIN], f32, name="vacc", tag="v",
                                   bufs=2)
                    for k in range(4):
                        nc.tensor.matmul(acc[:], mm(xt[k][:, tsl], proj_dt),
                                         mm(wv[k][:], proj_dt),
                                         start=(k == 0), stop=(k == 3))
                    nc.vector.scalar_tensor_tensor(vt[t][:], acc[:], 0.0,
                                                   bvb[:TT, :], op0=ALU.add,
                                                   op1=ALU.add)
                    nc.vector.tensor_scalar_max(vt[t][:], vt[t][:], 0.0)

                if stage == "proj":
                    for t in range(NT):
                        nc.vector.tensor_copy(ot[t][:], vt[t][:])
                        nc.sync.dma_start(out=out_d[t * TT:(t + 1) * TT, :],
                                          in_=ot[t][:])

            # ---- attention ----
            # Row-group safety: every PSUM bank only ever receives matmuls
            # from ONE base-partition (0 or 64).  Even heads (bp0) and odd
            # heads (bp64) accumulate in separate banks so the PE's
            # row-group-concurrent matmuls never co-write a bank.
            def attn_scores(dst, lhs_of, rhs_of, pair, tsl):
                # dst[0]: heads pair[0::2] at bp0; dst[1]: pair[1::2] at bp64
                nc.tensor.matmul(dst[0][:], ma[0:5, :], mb2[0:5, :],
                                 start=True, stop=False,
                                 skip_group_check=True)
                nc.tensor.matmul(dst[1][:], ma[64:69, :], mb2[64:69, :],
                                 start=True, stop=False,
                                 skip_group_check=True)
                for i, h in enumerate(pair):
                    ft, po = h // 2, (h % 2) * 64
                    par, col = h % 2, (i // 2) * TT
                    nc.tensor.matmul(
                        dst[par][:, col:col + TT],
                        mm(lhs_of[ft][po:po + 64, tsl], qk_dt),
                        mm(rhs_of[ft][po:po + 64, tsl], qk_dt),
                        start=False, stop=(i >= len(pair) - 2),
                        skip_group_check=True)

            if stage != "proj":
                ctx_aps = tc.tile_pool(name="apsum", bufs=4, space="PSUM")
                ctx_pvs = tc.tile_pool(name="pvpsum", bufs=2, space="PSUM")
                aps = ctx_aps.__enter__()
                pvs = ctx_pvs.__enter__()

            for t in range(NT) if stage != "proj" else []:
                tsl = slice(t * TT, (t + 1) * TT)
                for hg in range(2):
                    heads = [hg * 4, hg * 4 + 1, hg * 4 + 2, hg * 4 + 3]
                    sE = aps.tile([TT, 2 * TT], f32, name="sE", tag="s",
                                  bufs=4)
                    sO = aps.tile([TT, 2 * TT], f32, name="sO", tag="s",
                                  bufs=4)
                    attn_scores((sE, sO), qt, kt_, heads, tsl)
                    eE = sp.tile([TT, 2 * TT], f32, name="eE", tag="e",
                                 bufs=2)
                    eO = sp.tile([TT, 2 * TT], f32, name="eO", tag="e",
                                 bufs=2)
                    nc.scalar.activation(eE[:], sE[:], AF.Exp)
                    nc.scalar.activation(eO[:], sO[:], AF.Exp)

                    if stage == "attn_s":
                        if hg == 0:
                            nc.vector.tensor_copy(ot[t][:, :2 * TT], eE[:])
                            nc.vector.tensor_copy(ot[t][:, 2 * TT:4 * TT],
                                                  eO[:])
                            nc.sync.dma_start(
                                out=out_d[t * TT:(t + 1) * TT, :],
                                in_=ot[t][:])
                        continue

                    stE = aps.tile([TT, 2 * TT], f32, name="stE", tag="s",
                                   bufs=4)
                    stO = aps.tile([TT, 2 * TT], f32, name="stO", tag="s",
                                   bufs=4)
                    attn_scores((stE, stO), kt_, qt, heads, tsl)
                    etE = sp.tile([TT, 2 * TT], io_dt(pv_dt), name="etE",
                                  tag="et", bufs=2)
                    etO = sp.tile([TT, 2 * TT], io_dt(pv_dt), name="etO",
                                  tag="et", bufs=2)
                    nc.scalar.activation(etE[:], stE[:], AF.Exp)
                    nc.scalar.activation(etO[:], stO[:], AF.Exp)

                    rsE = sp.tile([TT, 2], f32, name="rsE", tag="rs", bufs=3)
                    rsO = sp.tile([TT, 2], f32, name="rsO", tag="rs", bufs=3)
                    nc.vector.tensor_reduce(
                        rsE[:], eE.rearrange("p (g k) -> p g k", g=2),
                        axis=AX.X, op=ALU.add)
                    nc.vector.tensor_reduce(
                        rsO[:], eO.rearrange("p (g k) -> p g k", g=2),
                        axis=AX.X, op=ALU.add)
                    rcE = sp.tile([TT, 2], f32, name="rcE", tag="rc", bufs=3)
                    rcO = sp.tile([TT, 2], f32, name="rcO", tag="rc", bufs=3)
                    nc.vector.reciprocal(rcE[:], rsE[:])
                    nc.vector.reciprocal(rcO[:], rsO[:])

                    for i, h in enumerate(heads):
                        par, col = h % 2, (i // 2) * TT
                        et = (etE, etO)[par]
                        rc = (rcE, rcO)[par]
                        pv = pvs.tile([TT, D], f32, name="pv", tag="pv",
                                      bufs=2)
                        nc.tensor.matmul(pv[:],
                                         mm(et[:, col:col + TT], pv_dt),
                                         mm(vt[t][:, h * D:(h + 1) * D],
                                            pv_dt),
                                         start=True, stop=True)
                        nc.scalar.activation(ot[t][:, h * D:(h + 1) * D],
                                             pv[:], AF.Copy,
                                             scale=rc[:, i // 2:i // 2 + 1])
                nc.sync.dma_start(out=out_d[tsl, :], in_=ot[t][:])

            if stage != "proj":
                ctx_pvs.__exit__(None, None, None)
                ctx_aps.__exit__(None, None, None)

    nc.compile()
    return nc


def _prep_inputs(x, Wq, bq, Wk, bk, Wv, bv, proj_dt):
    import ml_dtypes

    x = np.asarray(x, np.float32)
    Wq = np.asarray(Wq, np.float32)
    bq = np.asarray(bq, np.float32)
    Wk = np.asarray(Wk, np.float32)
    bk = np.asarray(bk, np.float32)
    Wv = np.asarray(Wv, np.float32)
    bv = np.asarray(bv, np.float32)

    scale = 1.0 / np.sqrt(np.float32(D))  # 1/8, exact
    wq_s = (Wq * scale).astype(np.float32)
    bq_s = (bq * scale).astype(np.float32)

    io_np = ml_dtypes.bfloat16 if proj_dt == "bf16" else np.float32
    xT = np.ascontiguousarray(x.transpose(0, 2, 1))  # [B, DIN, N]

    bqc = np.ascontiguousarray(bq_s.reshape(4, 128).T)
    bkc = np.ascontiguousarray(bk.reshape(4, 128).T)
    bvb = np.ascontiguousarray(np.tile(bv[None, :], (128, 1)))

    # rank-5 factors of the additive frame mask over one 100-token tile,
    # replicated at partition rows 0-4 (bp0 banks) and 64-68 (bp64 banks)
    mA = np.zeros((128, TT), ml_dtypes.bfloat16)
    mB = np.zeros((128, TT), ml_dtypes.bfloat16)
    big = ml_dtypes.bfloat16(9e15)
    for base in (0, 64):
        mA[base, :] = 1
        mB[base, :] = -big
        for f in range(4):
            mA[base + 1 + f, f * JN:(f + 1) * JN] = 1
            mB[base + 1 + f, f * JN:(f + 1) * JN] = big
    mB2 = np.ascontiguousarray(np.tile(mB, (1, 2)))

    in_maps = []
    for c in range(NCORES):
        b, fb = c // 4, c % 4
        in_maps.append({
            "xT": np.ascontiguousarray(
                xT[b, :, fb * TOK:(fb + 1) * TOK]).astype(io_np),
            "wq": wq_s.astype(io_np),
            "wk": Wk.astype(io_np),
            "wv": Wv.astype(io_np),
            "bqc": bqc, "bkc": bkc, "bvb": bvb,
            "mA": mA, "mB2": mB2,
        })
    return in_maps


def kernel(x, Wq, bq, Wk, bk, Wv, bv, att_heads=H, latent_dim=D,
           time_len=TL, joint_num=JN, **_):
    from concourse.bass_utils import run_bass_kernel_spmd

    cfg = tuple(sorted(CONFIG.items()))
    if cfg not in _CACHE:
        _CACHE[cfg] = _build(CONFIG)
    nc = _CACHE[cfg]

    in_maps = _prep_inputs(x, Wq, bq, Wk, bk, Wv, bv, CONFIG["proj"])
    res = run_bass_kernel_spmd(nc, in_maps, core_ids=list(range(NCORES)))
    global LAST_RESULT
    LAST_RESULT = res

    out = np.empty((B, N, DIN), np.float32)
    for c in range(NCORES):
        b, fb = c // 4, c % 4
        out[b, fb * TOK:(fb + 1) * TOK, :] = res.results[c]["out"]
    return out


# revision 17
# speedup vs baseline: 1.6795x; 1.6795x over previous
"""Block-diagonal (per-frame) multi-head attention on 8 Trainium2 cores.

Problem: x[2,3200,512] -> QKV proj (H=8 heads, D=64) -> attention masked to
25-token frames (128 frames) -> out[2,3200,512].  N = 3200 = 128*25.

Sharding: 256 (batch, frame) groups; core c handles batch c//4, frames
(c%4)*32..+32  => 800 tokens/core, tiled as 8 x 100 tokens (4 frames).

Layout trick: host sends x pre-transposed (xT [512, 800]) so every matmul
contracts over the partition dim:
  qT/kT [feat, tok] = W.T @ xT   (lhsT = W slice, rhs = xT)
  v     [tok, feat] = xT.T @ Wv  (lhsT = xT slice, rhs = Wv)
Scores per (head, tile): S = qT_h.T @ kT_h and S^T = kT_h.T @ qT_h -- both
directly available, no transposes anywhere.  The -9e15 frame mask is rank-5
(ones + 4 frame indicators), injected by one small matmul that initializes
the PSUM accumulation group.  softmax skips max-subtraction (|scores| <~ 8).
PV uses E^T = exp(S^T) as the stationary operand with v natural as moving.
"""

import numpy as np

B, N, DIN = 2, 3200, 512
H, D = 8, 64
TL, JN = 128, 25
NCORES = 8
TOK = 800      # tokens per core
NT = 8         # token tiles per core
TT = 100       # tokens per tile (4 frames)
NEG = -9e15

# matmul dtype per stage: 'f32' | 'f32r' | 'bf16'
#   proj: QKV projection matmuls (and dtype of xT/W in SBUF+HBM)
#   qk:   dtype of qT/kT tiles (scores matmuls)
#   pv:   dtype of E^T and V tiles (PV matmul)
CONFIG = {"proj": "f32", "qk": "f32", "pv": "f32"}

_CACHE = {}
LAST_RESULT = None  # BassKernelResults of the most recent kernel() call


def _build(cfg, stage="full"):
    import concourse.bacc as bacc
    import concourse.tile as tile
    from concourse import mybir

    f32 = mybir.dt.float32
    bf16 = mybir.dt.bfloat16
    f32r = mybir.dt.float32r
    AF = mybir.ActivationFunctionType
    ALU = mybir.AluOpType
    AX = mybir.AxisListType

    def io_dt(kind):
        return {"f32": f32, "f32r": f32r, "bf16": bf16}[kind]

    def mm(ap, kind):
        return ap

    proj_dt, qk_dt, pv_dt = cfg["proj"], cfg["qk"], cfg["pv"]

    nc = bacc.Bacc("TRN2", target_bir_lowering=False, debug=False,
                   num_devices=NCORES)

    xt_d = nc.dram_tensor("xT", [DIN, TOK], io_dt(proj_dt),
                          kind="ExternalInput").ap()
    w_d = {}
    for nm in ("wq", "wk", "wv"):
        w_d[nm] = nc.dram_tensor(nm, [DIN, DIN], io_dt(proj_dt),
                                 kind="ExternalInput").ap()
    bqc_d = nc.dram_tensor("bqc", [128, 4], f32, kind="ExternalInput").ap()
    bkc_d = nc.dram_tensor("bkc", [128, 4], f32, kind="ExternalInput").ap()
    bvb_d = nc.dram_tensor("bvb", [128, DIN], f32, kind="ExternalInput").ap()
    ma_d = nc.dram_tensor("mA", [128, TT], bf16, kind="ExternalInput").ap()
    mb2_d = nc.dram_tensor("mB2", [128, 2 * TT], bf16, kind="ExternalInput").ap()
    out_d = nc.dram_tensor("out", [TOK, DIN], f32, kind="ExternalOutput").ap()

    with tile.TileContext(nc) as tc:
        with (
            tc.tile_pool(name="persist", bufs=1) as pp,
            tc.tile_pool(name="scratch", bufs=2) as sp,
        ):
            # ---- DMA in (emission order ~ priority) ----
            wq = [pp.tile([128, DIN], io_dt(proj_dt), name=f"wq{k}",
                          tag=f"wq{k}") for k in range(4)]
            xt = [pp.tile([128, TOK], io_dt(proj_dt), name=f"xt{k}",
                          tag=f"xt{k}") for k in range(4)]
            for k in range(4):
                nc.sync.dma_start(out=wq[k], in_=w_d["wq"][k * 128:(k + 1) * 128, :])
                nc.sync.dma_start(out=xt[k], in_=xt_d[k * 128:(k + 1) * 128, :])
            bqc = pp.tile([128, 4], f32, name="bqc", tag="bqc")
            bkc = pp.tile([128, 4], f32, name="bkc", tag="bkc")
            nc.sync.dma_start(out=bqc, in_=bqc_d)
            nc.sync.dma_start(out=bkc, in_=bkc_d)
            wk = [pp.tile([128, DIN], io_dt(proj_dt), name=f"wk{k}",
                          tag=f"wk{k}") for k in range(4)]
            for k in range(4):
                nc.sync.dma_start(out=wk[k], in_=w_d["wk"][k * 128:(k + 1) * 128, :])
            wv = [pp.tile([128, DIN], io_dt(proj_dt), name=f"wv{k}",
                          tag=f"wv{k}") for k in range(4)]
            for k in range(4):
                nc.sync.dma_start(out=wv[k], in_=w_d["wv"][k * 128:(k + 1) * 128, :])
            bvb = pp.tile([128, DIN], f32, name="bvb", tag="bvb")
            nc.sync.dma_start(out=bvb, in_=bvb_d)
            ma = pp.tile([128, TT], bf16, name="ma", tag="ma")
            mb2 = pp.tile([128, 2 * TT], bf16, name="mb2", tag="mb2")
            nc.sync.dma_start(out=ma, in_=ma_d)
            nc.sync.dma_start(out=mb2, in_=mb2_d)

            # ---- persistent activations ----
            qt = [pp.tile([128, TOK], io_dt(qk_dt), name=f"qt{k}",
                          tag=f"qt{k}") for k in range(4)]
            kt_ = [pp.tile([128, TOK], io_dt(qk_dt), name=f"kt{k}",
                           tag=f"kt{k}") for k in range(4)]
            vt = [pp.tile([TT, DIN], io_dt(pv_dt), name=f"vt{t}",
                          tag=f"vt{t}") for t in range(NT)]
            ot = [pp.tile([TT, DIN], f32, name=f"ot{t}", tag=f"ot{t}")
                  for t in range(NT)]

            with (
                tc.tile_pool(name="ppsum", bufs=2, space="PSUM") as pps,
                tc.tile_pool(name="vpsum", bufs=2, space="PSUM") as vps,
            ):
                # ---- q^T / k^T projections: psum[feat, tok] ----
                for (w, bc, dst) in ((wq, bqc, qt), (wk, bkc, kt_)):
                    for ft in range(4):
                        fsl = slice(ft * 128, (ft + 1) * 128)
                        for ch in range(2):
                            csl = slice(ch * 400, (ch + 1) * 400)
                            acc = pps.tile([128, 400], f32, name="pacc",
                                           tag="p", bufs=2)
                            for k in range(4):
                                nc.tensor.matmul(
                                    acc[:], mm(w[k][:, fsl], proj_dt),
                                    mm(xt[k][:, csl], proj_dt),
                                    start=(k == 0), stop=(k == 3))
                            nc.scalar.activation(dst[ft][:, csl], acc[:],
                                                 AF.Identity,
                                                 bias=bc[:, ft:ft + 1])

                # ---- v projection: psum[tok, feat]; bias+relu on DVE ----
                for t in range(NT):
                    tsl = slice(t * TT, (t + 1) * TT)
                    acc = vps.tile([TT, DIN], f32, name="vacc", tag="v",
                                   bufs=2)
                    for k in range(4):
                        nc.tensor.matmul(acc[:], mm(xt[k][:, tsl], proj_dt),
                                         mm(wv[k][:], proj_dt),
                                         start=(k == 0), stop=(k == 3))
                    nc.vector.scalar_tensor_tensor(vt[t][:], acc[:], 0.0,
                                                   bvb[:TT, :], op0=ALU.add,
                                                   op1=ALU.add)
                    nc.vector.tensor_scalar_max(vt[t][:], vt[t][:], 0.0)

                if stage == "proj":
                    for t in range(NT):
                        nc.vector.tensor_copy(ot[t][:], vt[t][:])
                        nc.sync.dma_start(out=out_d[t * TT:(t + 1) * TT, :],
                                          in_=ot[t][:])

            # ---- attention ----
            # Row-group safety: every PSUM bank only ever receives matmuls
            # from ONE base-partition (0 or 64).  Even heads (bp0) and odd
            # heads (bp64) accumulate in separate banks so the PE's
            # row-group-concurrent matmuls never co-write a bank.
            def attn_scores(dst, lhs_of, rhs_of, pair, tsl):
                # dst[0]: heads pair[0::2] at bp0; dst[1]: pair[1::2] at bp64
                nc.tensor.matmul(dst[0][:], ma[0:5, :], mb2[0:5, :],
                                 start=True, stop=False,
                                 skip_group_check=True)
                nc.tensor.matmul(dst[1][:], ma[64:69, :], mb2[64:69, :],
                                 start=True, stop=False,
                                 skip_group_check=True)
                for i, h in enumerate(pair):
                    ft, po = h // 2, (h % 2) * 64
                    par, col = h % 2, (i // 2) * TT
                    nc.tensor.matmul(
                        dst[par][:, col:col + TT],
                        mm(lhs_of[ft][po:po + 64, tsl], qk_dt),
                        mm(rhs_of[ft][po:po + 64, tsl], qk_dt),
                        start=False, stop=(i >= len(pair) - 2),
                        skip_group_check=True)

            if stage != "proj":
                ctx_aps = tc.tile_pool(name="apsum", bufs=4, space="PSUM")
                ctx_pvs = tc.tile_pool(name="pvpsum", bufs=2, space="PSUM")
                aps = ctx_aps.__enter__()
                pvs = ctx_pvs.__enter__()

            for t in range(NT) if stage != "proj" else []:
                tsl = slice(t * TT, (t + 1) * TT)
                for hg in range(2):
                    heads = [hg * 4, hg * 4 + 1, hg * 4 + 2, hg * 4 + 3]
                    sE = aps.tile([TT, 2 * TT], f32, name="sE", tag="s",
                                  bufs=4)
                    sO = aps.tile([TT, 2 * TT], f32, name="sO", tag="s",
                                  bufs=4)
                    attn_scores((sE, sO), qt, kt_, heads, tsl)
                    eE = sp.tile([TT, 2 * TT], f32, name="eE", tag="e",
                                 bufs=2)
                    eO = sp.tile([TT, 2 * TT], f32, name="eO", tag="e",
                                 bufs=2)
                    nc.scalar.activation(eE[:], sE[:], AF.Exp)
                    nc.scalar.activation(eO[:], sO[:], AF.Exp)

                    if stage == "attn_s":
                        if hg == 0:
                            nc.vector.tensor_copy(ot[t][:, :2 * TT], eE[:])
                            nc.vector.tensor_copy(ot[t][:, 2 * TT:4 * TT],
                                                  eO[:])
                            nc.sync.dma_start(
                                out=out_d[t * TT:(t + 1) * TT, :],
                                in_=ot[t][:])
                        continue

                    stE = aps.tile([TT, 2 * TT], f32, name="stE", tag="s",
                                   bufs=4)
                    stO = aps.tile([TT, 2 * TT], f32, name="stO", tag="s",
                                   bufs=4)
                    attn_scores((stE, stO), kt_, qt, heads, tsl)
                    etE = sp.tile([TT, 2 * TT], io_dt(pv_dt), name="etE",
                                  tag="et", bufs=2)
                    etO = sp.tile([TT, 2 * TT], io_dt(pv_dt), name="etO",
                                  tag="et", bufs=2)
                    nc.scalar.activation(etE[:], stE[:], AF.Exp)
                    nc.scalar.activation(etO[:], stO[:], AF.Exp)

                    rsE = sp.tile([TT, 2], f32, name="rsE", tag="rs", bufs=3)
                    rsO = sp.tile([TT, 2], f32, name="rsO", tag="rs", bufs=3)
                    nc.vector.tensor_reduce(
                        rsE[:], eE.rearrange("p (g k) -> p g k", g=2),
                        axis=AX.X, op=ALU.add)
                    nc.vector.tensor_reduce(
                        rsO[:], eO.rearrange("p (g k) -> p g k", g=2),
                        axis=AX.X, op=ALU.add)
                    rcE = sp.tile([TT, 2], f32, name="rcE", tag="rc", bufs=3)
                    rcO = sp.tile([TT, 2], f32, name="rcO", tag="rc", bufs=3)
                    nc.vector.reciprocal(rcE[:], rsE[:])
                    nc.vector.reciprocal(rcO[:], rsO[:])

                    for i, h in enumerate(heads):
                        par, col = h % 2, (i // 2) * TT
                        et = (etE, etO)[par]
                        rc = (rcE, rcO)[par]
                        pv = pvs.tile([TT, D], f32, name="pv", tag="pv",
                                      bufs=2)
                        nc.tensor.matmul(pv[:],
                                         mm(et[:, col:col + TT], pv_dt),
                                         mm(vt[t][:, h * D:(h + 1) * D],
                                            pv_dt),
                                         start=True, stop=True)
                        nc.scalar.activation(ot[t][:, h * D:(h + 1) * D],
                                             pv[:], AF.Copy,
                                             scale=rc[:, i // 2:i // 2 + 1])
                nc.sync.dma_start(out=out_d[tsl, :], in_=ot[t][:])

            if stage != "proj":
                ctx_pvs.__exit__(None, None, None)
                ctx_aps.__exit__(None, None, None)

    nc.compile()
    return nc


def _prep_inputs(x, Wq, bq, Wk, bk, Wv, bv, proj_dt):
    import ml_dtypes

    x = np.asarray(x, np.float32)
    Wq = np.asarray(Wq, np.float32)
    bq = np.asarray(bq, np.float32)
    Wk = np.asarray(Wk, np.float32)
    bk = np.asarray(bk, np.float32)
    Wv = np.asarray(Wv, np.float32)
    bv = np.asarray(bv, np.float32)

    scale = 1.0 / np.sqrt(np.float32(D))  # 1/8, exact
    wq_s = (Wq * scale).astype(np.float32)
    bq_s = (bq * scale).astype(np.float32)

    io_np = ml_dtypes.bfloat16 if proj_dt == "bf16" else np.float32
    xT = np.ascontiguousarray(x.transpose(0, 2, 1))  # [B, DIN, N]

    bqc = np.ascontiguousarray(bq_s.reshape(4, 128).T)
    bkc = np.ascontiguousarray(bk.reshape(4, 128).T)
    bvb = np.ascontiguousarray(np.tile(bv[None, :], (128, 1)))

    # rank-5 factors of the additive frame mask over one 100-token tile,
    # replicated at partition rows 0-4 (bp0 banks) and 64-68 (bp64 banks)
    mA = np.zeros((128, TT), ml_dtypes.bfloat16)
    mB = np.zeros((128, TT), ml_dtypes.bfloat16)
    big = ml_dtypes.bfloat16(9e15)
    for base in (0, 64):
        mA[base, :] = 1
        mB[base, :] = -big
        for f in range(4):
            mA[base + 1 + f, f * JN:(f + 1) * JN] = 1
            mB[base + 1 + f, f * JN:(f + 1) * JN] = big
    mB2 = np.ascontiguousarray(np.tile(mB, (1, 2)))

    in_maps = []
    for c in range(NCORES):
        b, fb = c // 4, c % 4
        in_maps.append({
            "xT": np.ascontiguousarray(
                xT[b, :, fb * TOK:(fb + 1) * TOK]).astype(io_np),
            "wq": wq_s.astype(io_np),
            "wk": Wk.astype(io_np),
            "wv": Wv.astype(io_np),
            "bqc": bqc, "bkc": bkc, "bvb": bvb,
            "mA": mA, "mB2": mB2,
        })
    return in_maps


def kernel(x, Wq, bq, Wk, bk, Wv, bv, att_heads=H, latent_dim=D,
           time_len=TL, joint_num=JN, **_):
    from concourse.bass_utils import run_bass_kernel_spmd

    cfg = tuple(sorted(CONFIG.items()))
    if cfg not in _CACHE:
        _CACHE[cfg] = _build(CONFIG)
    nc = _CACHE[cfg]

    in_maps = _prep_inputs(x, Wq, bq, Wk, bk, Wv, bv, CONFIG["proj"])
    res = run_bass_kernel_spmd(nc, in_maps, core_ids=list(range(NCORES)))
    global LAST_RESULT
    LAST_RESULT = res

    out = np.empty((B, N, DIN), np.float32)
    for c in range(NCORES):
        b, fb = c // 4, c % 4
        out[b, fb * TOK:(fb + 1) * TOK, :] = res.results[c]["out"]
    return out


# revision 22
# speedup vs baseline: 1.9040x; 1.1337x over previous
"""Block-diagonal (per-frame) multi-head attention on 8 Trainium2 cores.

Problem: x[2,3200,512] -> QKV proj (H=8 heads, D=64) -> attention masked to
25-token frames (128 frames) -> out[2,3200,512].  N = 3200 = 128*25.

Sharding: 256 (batch, frame) groups; core c handles batch c//4, frames
(c%4)*32..+32  => 800 tokens/core, tiled as 8 x 100 tokens (4 frames).

Layout trick: host sends x pre-transposed (xT [512, 800]) so every matmul
contracts over the partition dim:
  qT/kT [feat, tok] = W.T @ xT   (lhsT = W slice, rhs = xT)
  v     [tok, feat] = xT.T @ Wv  (lhsT = xT slice, rhs = Wv)
Scores per (head, tile): S = qT_h.T @ kT_h and S^T = kT_h.T @ qT_h -- both
directly available, no transposes anywhere.  The -9e15 frame mask is rank-5
(ones + 4 frame indicators), injected by one small matmul that initializes
the PSUM accumulation group.  softmax skips max-subtraction (|scores| <~ 8).
PV uses E^T = exp(S^T) as the stationary operand with v natural as moving.
"""

import numpy as np

B, N, DIN = 2, 3200, 512
H, D = 8, 64
TL, JN = 128, 25
NCORES = 8
TOK = 800      # tokens per core
NT = 8         # token tiles per core
TT = 100       # tokens per tile (4 frames)
NEG = -9e15

# matmul dtype per stage: 'f32' | 'f32r' | 'bf16'
#   proj: QKV projection matmuls (and dtype of xT/W in SBUF+HBM)
#   qk:   dtype of qT/kT tiles (scores matmuls)
#   pv:   dtype of E^T and V tiles (PV matmul)
CONFIG = {"proj": "f32", "qk": "f32", "pv": "f32"}

_CACHE = {}
LAST_RESULT = None  # BassKernelResults of the most recent kernel() call


def _build(cfg, stage="full"):
    import concourse.bacc as bacc
    import concourse.tile as tile
    from concourse import mybir

    f32 = mybir.dt.float32
    bf16 = mybir.dt.bfloat16
    f32r = mybir.dt.float32r
    AF = mybir.ActivationFunctionType
    ALU = mybir.AluOpType
    AX = mybir.AxisListType

    def io_dt(kind):
        return {"f32": f32, "f32r": f32r, "bf16": bf16}[kind]

    def mm(ap, kind):
        return ap

    proj_dt, qk_dt, pv_dt = cfg["proj"], cfg["qk"], cfg["pv"]

    nc = bacc.Bacc("TRN2", target_bir_lowering=False, debug=False,
                   num_devices=NCORES)

    xt_d = nc.dram_tensor("xT", [DIN, TOK], io_dt(proj_dt),
                          kind="ExternalInput").ap()
    w_d = {}
    for nm in ("wq", "wk", "wv"):
        w_d[nm] = nc.dram_tensor(nm, [DIN, DIN], io_dt(proj_dt),
                                 kind="ExternalInput").ap()
    bqc_d = nc.dram_tensor("bqc", [128, 4], f32, kind="ExternalInput").ap()
    bkc_d = nc.dram_tensor("bkc", [128, 4], f32, kind="ExternalInput").ap()
    bvb_d = nc.dram_tensor("bvb", [128, DIN], f32, kind="ExternalInput").ap()
    ma_d = nc.dram_tensor("mA", [128, TT], bf16, kind="ExternalInput").ap()
    mb2_d = nc.dram_tensor("mB2", [128, 2 * TT], bf16, kind="ExternalInput").ap()
    out_d = nc.dram_tensor("out", [TOK, DIN], f32, kind="ExternalOutput").ap()

    with tile.TileContext(nc) as tc:
        with (
            tc.tile_pool(name="persist", bufs=1) as pp,
            tc.tile_pool(name="scratch", bufs=2) as sp,
        ):
            # ---- DMA in (emission order ~ priority) ----
            wq = [pp.tile([128, DIN], io_dt(proj_dt), name=f"wq{k}",
                          tag=f"wq{k}") for k in range(4)]
            xt = [pp.tile([128, TOK], io_dt(proj_dt), name=f"xt{k}",
                          tag=f"xt{k}") for k in range(4)]
            for k in range(4):
                nc.sync.dma_start(out=wq[k], in_=w_d["wq"][k * 128:(k + 1) * 128, :])
                nc.sync.dma_start(out=xt[k], in_=xt_d[k * 128:(k + 1) * 128, :])
            bqc = pp.tile([128, 4], f32, name="bqc", tag="bqc")
            bkc = pp.tile([128, 4], f32, name="bkc", tag="bkc")
            nc.sync.dma_start(out=bqc, in_=bqc_d)
            nc.sync.dma_start(out=bkc, in_=bkc_d)
            wk = [pp.tile([128, DIN], io_dt(proj_dt), name=f"wk{k}",
                          tag=f"wk{k}") for k in range(4)]
            for k in range(4):
                nc.sync.dma_start(out=wk[k], in_=w_d["wk"][k * 128:(k + 1) * 128, :])
            wv = [pp.tile([128, DIN], io_dt(proj_dt), name=f"wv{k}",
                          tag=f"wv{k}") for k in range(4)]
            for k in range(4):
                nc.sync.dma_start(out=wv[k], in_=w_d["wv"][k * 128:(k + 1) * 128, :])
            bvb = pp.tile([128, DIN], f32, name="bvb", tag="bvb")
            nc.sync.dma_start(out=bvb, in_=bvb_d)
            ma = pp.tile([128, TT], bf16, name="ma", tag="ma")
            mb2 = pp.tile([128, 2 * TT], bf16, name="mb2", tag="mb2")
            nc.sync.dma_start(out=ma, in_=ma_d)
            nc.sync.dma_start(out=mb2, in_=mb2_d)

            # ---- persistent activations ----
            qt = [pp.tile([128, TOK], io_dt(qk_dt), name=f"qt{k}",
                          tag=f"qt{k}") for k in range(4)]
            kt_ = [pp.tile([128, TOK], io_dt(qk_dt), name=f"kt{k}",
                           tag=f"kt{k}") for k in range(4)]
            vt = [pp.tile([TT, DIN], io_dt(pv_dt), name=f"vt{t}",
                          tag=f"vt{t}") for t in range(NT)]
            ot = [pp.tile([TT, DIN], f32, name=f"ot{t}", tag=f"ot{t}")
                  for t in range(NT)]

            with (
                tc.tile_pool(name="ppsum", bufs=2, space="PSUM") as pps,
                tc.tile_pool(name="vpsum", bufs=2, space="PSUM") as vps,
            ):
                # ---- q^T / k^T projections: psum[feat, tok] ----
                for (w, bc, dst) in ((wq, bqc, qt), (wk, bkc, kt_)):
                    for ft in range(4):
                        fsl = slice(ft * 128, (ft + 1) * 128)
                        for ch in range(2):
                            csl = slice(ch * 400, (ch + 1) * 400)
                            acc = pps.tile([128, 400], f32, name="pacc",
                                           tag="p", bufs=2)
                            for k in range(4):
                                nc.tensor.matmul(
                                    acc[:], mm(w[k][:, fsl], proj_dt),
                                    mm(xt[k][:, csl], proj_dt),
                                    start=(k == 0), stop=(k == 3))
                            nc.scalar.activation(dst[ft][:, csl], acc[:],
                                                 AF.Identity,
                                                 bias=bc[:, ft:ft + 1])

                # ---- v projection: psum[tok, feat]; bias+relu on DVE ----
                for t in range(NT):
                    tsl = slice(t * TT, (t + 1) * TT)
                    acc = vps.tile([TT, DIN], f32, name="vacc", tag="v",
                                   bufs=2)
                    for k in range(4):
                        nc.tensor.matmul(acc[:], mm(xt[k][:, tsl], proj_dt),
                                         mm(wv[k][:], proj_dt),
                                         start=(k == 0), stop=(k == 3))
                    nc.vector.scalar_tensor_tensor(vt[t][:], acc[:], 0.0,
                                                   bvb[:TT, :], op0=ALU.add,
                                                   op1=ALU.add)
                    nc.vector.tensor_scalar_max(vt[t][:], vt[t][:], 0.0)

                if stage == "proj":
                    for t in range(NT):
                        nc.vector.tensor_copy(ot[t][:], vt[t][:])
                        nc.sync.dma_start(out=out_d[t * TT:(t + 1) * TT, :],
                                          in_=ot[t][:])

            # ---- attention ----
            # Row-group safety: every PSUM bank only ever receives matmuls
            # from ONE base-partition (0 or 64).  Even heads (bp0) and odd
            # heads (bp64) accumulate in separate banks so the PE's
            # row-group-concurrent matmuls never co-write a bank.
            def attn_scores(dst, lhs_of, rhs_of, pair, tsl):
                # dst[0]: heads pair[0::2] at bp0; dst[1]: pair[1::2] at bp64
                nc.tensor.matmul(dst[0][:], ma[0:5, :], mb2[0:5, :],
                                 start=True, stop=False,
                                 skip_group_check=True)
                nc.tensor.matmul(dst[1][:], ma[64:69, :], mb2[64:69, :],
                                 start=True, stop=False,
                                 skip_group_check=True)
                for i, h in enumerate(pair):
                    ft, po = h // 2, (h % 2) * 64
                    par, col = h % 2, (i // 2) * TT
                    nc.tensor.matmul(
                        dst[par][:, col:col + TT],
                        mm(lhs_of[ft][po:po + 64, tsl], qk_dt),
                        mm(rhs_of[ft][po:po + 64, tsl], qk_dt),
                        start=False, stop=(i >= len(pair) - 2),
                        skip_group_check=True)

            if stage != "proj":
                ctx_aps = tc.tile_pool(name="apsum", bufs=6, space="PSUM")
                aps = ctx_aps.__enter__()
                pvs = aps

            for t in range(NT) if stage != "proj" else []:
                tsl = slice(t * TT, (t + 1) * TT)
                for hg in range(2):
                    heads = [hg * 4, hg * 4 + 1, hg * 4 + 2, hg * 4 + 3]
                    sE = aps.tile([TT, 2 * TT], f32, name="sE", tag="s",
                                  bufs=6)
                    sO = aps.tile([TT, 2 * TT], f32, name="sO", tag="s",
                                  bufs=6)
                    attn_scores((sE, sO), qt, kt_, heads, tsl)
                    eE = sp.tile([TT, 2 * TT], f32, name="eE", tag="e",
                                 bufs=4)
                    eO = sp.tile([TT, 2 * TT], f32, name="eO", tag="e",
                                 bufs=4)
                    if stage == "attn_s":
                        nc.scalar.activation(eE[:], sE[:], AF.Exp)
                        nc.scalar.activation(eO[:], sO[:], AF.Exp)
                        if hg == 0:
                            nc.vector.tensor_copy(ot[t][:, :2 * TT], eE[:])
                            nc.vector.tensor_copy(ot[t][:, 2 * TT:4 * TT],
                                                  eO[:])
                            nc.sync.dma_start(
                                out=out_d[t * TT:(t + 1) * TT, :],
                                in_=ot[t][:])
                        continue

                    stE = aps.tile([TT, 2 * TT], f32, name="stE", tag="s",
                                   bufs=6)
                    stO = aps.tile([TT, 2 * TT], f32, name="stO", tag="s",
                                   bufs=6)
                    attn_scores((stE, stO), kt_, qt, heads, tsl)
                    etE = sp.tile([TT, 2 * TT], io_dt(pv_dt), name="etE",
                                  tag="et", bufs=4)
                    etO = sp.tile([TT, 2 * TT], io_dt(pv_dt), name="etO",
                                  tag="et", bufs=4)
                    nc.scalar.activation(eE[:], sE[:], AF.Exp)
                    nc.scalar.activation(eO[:], sO[:], AF.Exp)
                    nc.scalar.activation(etE[:], stE[:], AF.Exp)
                    nc.scalar.activation(etO[:], stO[:], AF.Exp)

                    rsE = sp.tile([TT, 2], f32, name="rsE", tag="rs", bufs=6)
                    rsO = sp.tile([TT, 2], f32, name="rsO", tag="rs", bufs=6)
                    nc.vector.tensor_reduce(
                        rsE[:], eE.rearrange("p (g k) -> p g k", g=2),
                        axis=AX.X, op=ALU.add)
                    nc.vector.tensor_reduce(
                        rsO[:], eO.rearrange("p (g k) -> p g k", g=2),
                        axis=AX.X, op=ALU.add)
                    rcE = sp.tile([TT, 2], f32, name="rcE", tag="rc", bufs=6)
                    rcO = sp.tile([TT, 2], f32, name="rcO", tag="rc", bufs=6)
                    nc.vector.reciprocal(rcE[:], rsE[:])
                    nc.vector.reciprocal(rcO[:], rsO[:])

                    for i, h in enumerate(heads):
                        par, col = h % 2, (i // 2) * TT
                        et = (etE, etO)[par]
                        rc = (rcE, rcO)[par]
                        pv = pvs.tile([TT, D], f32, name="pv", tag="pv",
                                      bufs=2)
                        nc.tensor.matmul(pv[:],
                                         mm(et[:, col:col + TT], pv_dt),
                                         mm(vt[t][:, h * D:(h + 1) * D],
                                            pv_dt),
                                         start=True, stop=True)
                        nc.vector.tensor_scalar_mul(
                            ot[t][:, h * D:(h + 1) * D], pv[:],
                            rc[:, i // 2:i // 2 + 1])
                nc.sync.dma_start(out=out_d[tsl, :], in_=ot[t][:])

            if stage != "proj":
                ctx_aps.__exit__(None, None, None)

    nc.compile()
    return nc


def _prep_inputs(x, Wq, bq, Wk, bk, Wv, bv, proj_dt):
    import ml_dtypes

    x = np.asarray(x, np.float32)
    Wq = np.asarray(Wq, np.float32)
    bq = np.asarray(bq, np.float32)
    Wk = np.asarray(Wk, np.float32)
    bk = np.asarray(bk, np.float32)
    Wv = np.asarray(Wv, np.float32)
    bv = np.asarray(bv, np.float32)

    scale = 1.0 / np.sqrt(np.float32(D))  # 1/8, exact
    wq_s = (Wq * scale).astype(np.float32)
    bq_s = (bq * scale).astype(np.float32)

    io_np = ml_dtypes.bfloat16 if proj_dt == "bf16" else np.float32
    xT = np.ascontiguousarray(x.transpose(0, 2, 1))  # [B, DIN, N]

    bqc = np.ascontiguousarray(bq_s.reshape(4, 128).T)
    bkc = np.ascontiguousarray(bk.reshape(4, 128).T)
    bvb = np.ascontiguousarray(np.tile(bv[None, :], (128, 1)))

    # rank-5 factors of the additive frame mask over one 100-token tile,
    # replicated at partition rows 0-4 (bp0 banks) and 64-68 (bp64 banks)
    mA = np.zeros((128, TT), ml_dtypes.bfloat16)
    mB = np.zeros((128, TT), ml_dtypes.bfloat16)
    big = ml_dtypes.bfloat16(9e15)
    for base in (0, 64):
        mA[base, :] = 1
        mB[base, :] = -big
        for f in range(4):
            mA[base + 1 + f, f * JN:(f + 1) * JN] = 1
            mB[base + 1 + f, f * JN:(f + 1) * JN] = big
    mB2 = np.ascontiguousarray(np.tile(mB, (1, 2)))

    in_maps = []
    for c in range(NCORES):
        b, fb = c // 4, c % 4
        in_maps.append({
            "xT": np.ascontiguousarray(
                xT[b, :, fb * TOK:(fb + 1) * TOK]).astype(io_np),
            "wq": wq_s.astype(io_np),
            "wk": Wk.astype(io_np),
            "wv": Wv.astype(io_np),
            "bqc": bqc, "bkc": bkc, "bvb": bvb,
            "mA": mA, "mB2": mB2,
        })
    return in_maps


def kernel(x, Wq, bq, Wk, bk, Wv, bv, att_heads=H, latent_dim=D,
           time_len=TL, joint_num=JN, **_):
    from concourse.bass_utils import run_bass_kernel_spmd

    cfg = tuple(sorted(CONFIG.items()))
    if cfg not in _CACHE:
        _CACHE[cfg] = _build(CONFIG)
    nc = _CACHE[cfg]

    in_maps = _prep_inputs(x, Wq, bq, Wk, bk, Wv, bv, CONFIG["proj"])
    res = run_bass_kernel_spmd(nc, in_maps, core_ids=list(range(NCORES)))
    global LAST_RESULT
    LAST_RESULT = res

    out = np.empty((B, N, DIN), np.float32)
    for c in range(NCORES):
        b, fb = c // 4, c % 4
        out[b, fb * TOK:(fb + 1) * TOK, :] = res.results[c]["out"]
    return out


# revision 29
# speedup vs baseline: 2.4592x; 1.2916x over previous
"""Block-diagonal (per-frame) multi-head attention on 8 Trainium2 cores.

Problem: x[2,3200,512] -> QKV proj (H=8 heads, D=64) -> attention masked to
25-token frames (128 frames) -> out[2,3200,512].  N = 3200 = 128*25.

Sharding: 256 (batch, frame) groups; core c handles batch c//4, frames
(c%4)*32..+32  => 800 tokens/core, tiled as 8 x 100 tokens (4 frames).

Layout trick: host sends x pre-transposed (xT [512, 800]) so every matmul
contracts over the partition dim:
  qT/kT [feat, tok] = W.T @ xT   (lhsT = W slice, rhs = xT)
  v     [tok, feat] = xT.T @ Wv  (lhsT = xT slice, rhs = Wv)
Scores per (head, tile): S = qT_h.T @ kT_h and S^T = kT_h.T @ qT_h -- both
directly available, no transposes anywhere.  The -9e15 frame mask is rank-5
(ones + 4 frame indicators), injected by one small matmul that initializes
the PSUM accumulation group.  softmax skips max-subtraction (|scores| <~ 8).
PV uses E^T = exp(S^T) as the stationary operand with v natural as moving.
"""

import numpy as np

B, N, DIN = 2, 3200, 512
H, D = 8, 64
TL, JN = 128, 25
NCORES = 8
TOK = 800      # tokens per core
NT = 8         # token tiles per core
TT = 100       # tokens per tile (4 frames)
NEG = -9e15

# matmul dtype per stage: 'f32' | 'f32r' | 'bf16'
#   proj: QKV projection matmuls (and dtype of xT/W in SBUF+HBM)
#   qk:   dtype of qT/kT tiles (scores matmuls)
#   pv:   dtype of E^T and V tiles (PV matmul)
CONFIG = {"proj": "f32", "qk": "f32", "pv": "f32"}

_CACHE = {}
LAST_RESULT = None  # BassKernelResults of the most recent kernel() call


def _build(cfg, stage="full"):
    import concourse.bacc as bacc
    import concourse.tile as tile
    from concourse import mybir

    f32 = mybir.dt.float32
    bf16 = mybir.dt.bfloat16
    f32r = mybir.dt.float32r
    AF = mybir.ActivationFunctionType
    ALU = mybir.AluOpType
    AX = mybir.AxisListType

    def io_dt(kind):
        return {"f32": f32, "f32r": f32r, "bf16": bf16}[kind]

    def mm(ap, kind):
        return ap

    proj_dt, qk_dt, pv_dt = cfg["proj"], cfg["qk"], cfg["pv"]

    nc = bacc.Bacc("TRN2", target_bir_lowering=False, debug=False,
                   num_devices=NCORES)

    xt_d = nc.dram_tensor("xT", [DIN, TOK], io_dt(proj_dt),
                          kind="ExternalInput").ap()
    w_d = {}
    for nm in ("wq", "wk", "wv"):
        w_d[nm] = nc.dram_tensor(nm, [DIN, DIN], io_dt(proj_dt),
                                 kind="ExternalInput").ap()
    bqc_d = nc.dram_tensor("bqc", [128, 4], f32, kind="ExternalInput").ap()
    bkc_d = nc.dram_tensor("bkc", [128, 4], f32, kind="ExternalInput").ap()
    bvb_d = nc.dram_tensor("bvb", [128, DIN], f32, kind="ExternalInput").ap()
    m01_d = nc.dram_tensor("m01", [TT, 2 * TT], bf16, kind="ExternalInput").ap()
    out_d = nc.dram_tensor("out", [TOK, DIN], f32, kind="ExternalOutput").ap()

    with tile.TileContext(nc) as tc:
        with (
            tc.tile_pool(name="persist", bufs=1) as pp,
            tc.tile_pool(name="scratch", bufs=2) as sp,
        ):
            # ---- DMA in (emission order ~ priority) ----
            wq = [pp.tile([128, DIN], io_dt(proj_dt), name=f"wq{k}",
                          tag=f"wq{k}") for k in range(4)]
            xt = [pp.tile([128, TOK], io_dt(proj_dt), name=f"xt{k}",
                          tag=f"xt{k}") for k in range(4)]
            for k in range(4):
                nc.sync.dma_start(out=wq[k], in_=w_d["wq"][k * 128:(k + 1) * 128, :])
                nc.sync.dma_start(out=xt[k], in_=xt_d[k * 128:(k + 1) * 128, :])
            bqc = pp.tile([128, 4], f32, name="bqc", tag="bqc")
            bkc = pp.tile([128, 4], f32, name="bkc", tag="bkc")
            nc.sync.dma_start(out=bqc, in_=bqc_d)
            nc.sync.dma_start(out=bkc, in_=bkc_d)
            wk = [pp.tile([128, DIN], io_dt(proj_dt), name=f"wk{k}",
                          tag=f"wk{k}") for k in range(4)]
            for k in range(4):
                nc.sync.dma_start(out=wk[k], in_=w_d["wk"][k * 128:(k + 1) * 128, :])
            wv = [pp.tile([128, DIN], io_dt(proj_dt), name=f"wv{k}",
                          tag=f"wv{k}") for k in range(4)]
            for k in range(4):
                nc.sync.dma_start(out=wv[k], in_=w_d["wv"][k * 128:(k + 1) * 128, :])
            bvb = pp.tile([128, DIN], f32, name="bvb", tag="bvb")
            nc.sync.dma_start(out=bvb, in_=bvb_d)
            m01 = pp.tile([TT, 2 * TT], bf16, name="m01", tag="m01")
            nc.sync.dma_start(out=m01, in_=m01_d)

            # ---- persistent activations ----
            qt = [pp.tile([128, TOK], io_dt(qk_dt), name=f"qt{k}",
                          tag=f"qt{k}") for k in range(4)]
            kt_ = [pp.tile([128, TOK], io_dt(qk_dt), name=f"kt{k}",
                           tag=f"kt{k}") for k in range(4)]
            # v with 65 columns per head: col h*65+64 is all-ones so the PV
            # matmul also produces the softmax denominator in its last column
            vt = [pp.tile([TT, H * (D + 1)], io_dt(pv_dt), name=f"vt{t}",
                          tag=f"vt{t}") for t in range(NT)]
            ot = [pp.tile([TT, DIN], f32, name=f"ot{t}", tag=f"ot{t}")
                  for t in range(NT)]

            with (
                tc.tile_pool(name="ppsum", bufs=2, space="PSUM") as pps,
                tc.tile_pool(name="vpsum", bufs=2, space="PSUM") as vps,
            ):
                # ---- q^T / k^T projections: psum[feat, tok] ----
                for (w, bc, dst) in ((wq, bqc, qt), (wk, bkc, kt_)):
                    for ft in range(4):
                        fsl = slice(ft * 128, (ft + 1) * 128)
                        for ch in range(2):
                            csl = slice(ch * 400, (ch + 1) * 400)
                            acc = pps.tile([128, 400], f32, name="pacc",
                                           tag="p", bufs=2)
                            for k in range(4):
                                nc.tensor.matmul(
                                    acc[:], mm(w[k][:, fsl], proj_dt),
                                    mm(xt[k][:, csl], proj_dt),
                                    start=(k == 0), stop=(k == 3))
                            nc.scalar.activation(dst[ft][:, csl], acc[:],
                                                 AF.Identity,
                                                 bias=bc[:, ft:ft + 1])

                # ---- v projection: psum[tok, feat]; bias+relu on DVE ----
                for t in range(NT):
                    tsl = slice(t * TT, (t + 1) * TT)
                    acc = vps.tile([TT, DIN], f32, name="vacc", tag="v",
                                   bufs=2)
                    for k in range(4):
                        nc.tensor.matmul(acc[:], mm(xt[k][:, tsl], proj_dt),
                                         mm(wv[k][:], proj_dt),
                                         start=(k == 0), stop=(k == 3))
                    vdat = vt[t].rearrange("p (h c) -> p h c", c=D + 1)[:, :, :D]
                    vones = vt[t].rearrange("p (h c) -> p h c",
                                            c=D + 1)[:, :, D:D + 1]
                    nc.vector.scalar_tensor_tensor(
                        vdat, acc.rearrange("p (h c) -> p h c", c=D), 0.0,
                        bvb[:TT, :].rearrange("p (h c) -> p h c", c=D),
                        op0=ALU.add, op1=ALU.add)
                    nc.vector.tensor_scalar_max(vdat, vdat, 0.0)
                    nc.vector.memset(vones, 1.0)

                if stage == "proj":
                    for t in range(NT):
                        nc.vector.tensor_copy(ot[t][:], vt[t][:])
                        nc.sync.dma_start(out=out_d[t * TT:(t + 1) * TT, :],
                                          in_=ot[t][:])

            # ---- attention ----
            # Only S^T = K_h^T-stationary @ Q_h is computed (per head, per
            # tile).  E^T = exp(S^T) ⊙ block-mask is the PV stationary; the
            # ones-column of v turns PV's last column into the softmax
            # denominator.  Row-group safety: each PSUM bank only receives
            # matmuls from ONE base-partition (0 or 64); even heads (bp0)
            # and odd heads (bp64) use separate banks so the PE's
            # row-group-concurrent matmuls never co-write a bank.
            if stage != "proj":
                ctx_aps = tc.tile_pool(name="apsum", bufs=6, space="PSUM")
                aps = ctx_aps.__enter__()

            for t in range(NT) if stage != "proj" else []:
                tsl = slice(t * TT, (t + 1) * TT)
                for hg in range(2):
                    heads = [hg * 4, hg * 4 + 1, hg * 4 + 2, hg * 4 + 3]
                    stE = aps.tile([TT, 2 * TT], f32, name="stE", tag="s",
                                   bufs=6)
                    stO = aps.tile([TT, 2 * TT], f32, name="stO", tag="s",
                                   bufs=6)
                    for i, h in enumerate(heads):
                        ft, po = h // 2, (h % 2) * 64
                        dst = (stE, stO)[h % 2]
                        col = (i // 2) * TT
                        nc.tensor.matmul(
                            dst[:, col:col + TT],
                            mm(kt_[ft][po:po + 64, tsl], qk_dt),
                            mm(qt[ft][po:po + 64, tsl], qk_dt),
                            start=(i < 2), stop=(i >= 2),
                            skip_group_check=True)
                    etE = sp.tile([TT, 2 * TT], io_dt(pv_dt), name="etE",
                                  tag="et", bufs=4)
                    etO = sp.tile([TT, 2 * TT], io_dt(pv_dt), name="etO",
                                  tag="et", bufs=4)
                    nc.scalar.activation(etE[:], stE[:], AF.Exp)
                    nc.scalar.activation(etO[:], stO[:], AF.Exp)
                    nc.vector.tensor_mul(etE[:], etE[:], m01[:])
                    nc.vector.tensor_mul(etO[:], etO[:], m01[:])

                    for i, h in enumerate(heads):
                        et = (etE, etO)[h % 2]
                        col = (i // 2) * TT
                        pv = aps.tile([TT, D + 1], f32, name="pv", tag="pv",
                                      bufs=2)
                        nc.tensor.matmul(pv[:],
                                         mm(et[:, col:col + TT], pv_dt),
                                         mm(vt[t][:, h * (D + 1):
                                                  (h + 1) * (D + 1)], pv_dt),
                                         start=True, stop=True)
                        rc = sp.tile([TT, 1], f32, name="rc", tag="rc",
                                     bufs=8)
                        nc.vector.reciprocal(rc[:], pv[:, D:D + 1])
                        nc.vector.tensor_scalar_mul(
                            ot[t][:, h * D:(h + 1) * D], pv[:, :D], rc[:])
                nc.sync.dma_start(out=out_d[tsl, :], in_=ot[t][:])

            if stage != "proj":
                ctx_aps.__exit__(None, None, None)

    nc.compile()
    return nc


def _prep_inputs(x, Wq, bq, Wk, bk, Wv, bv, proj_dt):
    import ml_dtypes

    x = np.asarray(x, np.float32)
    Wq = np.asarray(Wq, np.float32)
    bq = np.asarray(bq, np.float32)
    Wk = np.asarray(Wk, np.float32)
    bk = np.asarray(bk, np.float32)
    Wv = np.asarray(Wv, np.float32)
    bv = np.asarray(bv, np.float32)

    scale = 1.0 / np.sqrt(np.float32(D))  # 1/8, exact
    wq_s = (Wq * scale).astype(np.float32)
    bq_s = (bq * scale).astype(np.float32)

    io_np = ml_dtypes.bfloat16 if proj_dt == "bf16" else np.float32
    xT = np.ascontiguousarray(x.transpose(0, 2, 1))  # [B, DIN, N]

    bqc = np.ascontiguousarray(bq_s.reshape(4, 128).T)
    bkc = np.ascontiguousarray(bk.reshape(4, 128).T)
    bvb = np.ascontiguousarray(np.tile(bv[None, :], (128, 1)))

    # 0/1 block-diagonal frame mask over one 100-token tile, two heads wide
    blk = np.kron(np.eye(4, dtype=np.float32), np.ones((JN, JN), np.float32))
    m01 = np.ascontiguousarray(np.tile(blk, (1, 2))).astype(ml_dtypes.bfloat16)

    in_maps = []
    for c in range(NCORES):
        b, fb = c // 4, c % 4
        in_maps.append({
            "xT": np.ascontiguousarray(
                xT[b, :, fb * TOK:(fb + 1) * TOK]).astype(io_np),
            "wq": wq_s.astype(io_np),
            "wk": Wk.astype(io_np),
            "wv": Wv.astype(io_np),
            "bqc": bqc, "bkc": bkc, "bvb": bvb,
            "m01": m01,
        })
    return in_maps


def kernel(x, Wq, bq, Wk, bk, Wv, bv, att_heads=H, latent_dim=D,
           time_len=TL, joint_num=JN, **_):
    from concourse.bass_utils import run_bass_kernel_spmd

    cfg = tuple(sorted(CONFIG.items()))
    if cfg not in _CACHE:
        _CACHE[cfg] = _build(CONFIG)
    nc = _CACHE[cfg]

    in_maps = _prep_inputs(x, Wq, bq, Wk, bk, Wv, bv, CONFIG["proj"])
    res = run_bass_kernel_spmd(nc, in_maps, core_ids=list(range(NCORES)))
    global LAST_RESULT
    LAST_RESULT = res

    out = np.empty((B, N, DIN), np.float32)
    for c in range(NCORES):
        b, fb = c // 4, c % 4
        out[b, fb * TOK:(fb + 1) * TOK, :] = res.results[c]["out"]
    return out
